# revision 1
# baseline (speedup 1.0000x reference)
"""Distributed 3-layer GraphConv (PyG GraphConv + ReLU + softmax) for
nn_DecoderSpin on 8 trn2 NeuronCores.

Sharding: nodes partitioned into 8 contiguous dst-shards (12500 nodes per
core); every core aggregates the incoming edges of its own shard. The tiny
weight matrices are replicated. Cross-shard source features are exchanged
between the three layer launches as full gather tables (host concatenates
the 8 per-core shard outputs; all arithmetic runs on device).

Per-core layer kernel (Bass/Tile):
  - ELL-format neighbor gather: host builds degree-sorted node tiles of 128
    with a global per-tile depth schedule K[t]; slot (lane, depth j) holds
    the gather-table row of the j-th neighbor (pad slots point at an
    always-zero row). Gathers use gpsimd indirect DMA, 128 rows/instruction.
  - Aggregation: DVE tensor_reduce over each tile's K columns.
  - Dense transforms: feature-major PE matmuls (lhsT = weight matrices),
    agg tiles transposed via PE; ReLU+bias on the scalar engine; final
    2-class softmax on DVE/ACT.

kernel(**inputs) takes the full unsharded inputs and returns the full
[100000, 2] float32 softmax output. LAST_EXEC_TIME_NS holds the summed
hardware execution time of the three launches when profiling is available.
"""

import sys

sys.path.insert(0, "/opt/trn_rl_repo")

import numpy as np
import concourse.bass as bass
import concourse.bacc as bacc
import concourse.mybir as mybir
import concourse.tile as tile
from concourse.bass_utils import run_bass_kernel_spmd
from concourse.masks import make_identity

P = 128
N_CORES = 8
N_NODES = 100000
F32 = mybir.dt.float32
I32 = mybir.dt.int32

LAST_EXEC_TIME_NS = None


def _install_ntff_hook():
    """Best-effort: make trace=True work under the axon agent env."""
    try:
        import types
        import antenv  # noqa: F401
        try:
            from antenv.axon_hooks import get_axon_ntff_profile_hook  # noqa: F401
            return True
        except Exception:
            pass
        from trn_agent_boot.trn_boot import _ntff_profile_via_ctypes
        hook = _ntff_profile_via_ctypes("/opt/axon/libaxon_pjrt.so")
        mod = types.ModuleType("antenv.axon_hooks")
        mod.set_axon_ntff_profile_hook = lambda h: None
        mod.get_axon_ntff_profile_hook = lambda: hook
        sys.modules["antenv.axon_hooks"] = mod
        antenv.axon_hooks = mod
        return hook is not None
    except Exception:
        return False


def preprocess(edge_index, n_nodes):
    S = n_nodes // N_CORES
    TPC = (S + P - 1) // P
    SPAD = TPC * P
    src = np.asarray(edge_index[0], dtype=np.int64)
    dst = np.asarray(edge_index[1], dtype=np.int64)

    perms = []
    pos_of = np.empty(n_nodes, dtype=np.int64)
    deg_sorted = np.zeros((N_CORES, SPAD), dtype=np.int64)
    core_edges = []
    for c in range(N_CORES):
        m = (dst >= c * S) & (dst < (c + 1) * S)
        s_c = src[m]
        d_c = dst[m] - c * S
        deg = np.bincount(d_c, minlength=S)
        order = np.argsort(-deg, kind="stable").astype(np.int64)
        inv = np.empty(S, dtype=np.int64)
        inv[order] = np.arange(S)
        pos_of[c * S:(c + 1) * S] = c * SPAD + inv
        perms.append(order)
        deg_sorted[c, :S] = deg[order]
        core_edges.append((s_c, d_c, inv))

    K = deg_sorted.reshape(N_CORES, TPC, P).max(axis=2).max(axis=0)
    col_of_tile = np.concatenate([[0], np.cumsum(K)]).astype(np.int64)
    L = int(col_of_tile[-1])

    ell = np.empty((N_CORES, P, L), dtype=np.int32)
    for c in range(N_CORES):
        ell[c, :, :] = c * SPAD + S  # own shard's first pad row (always zero)
        s_c, d_c, inv = core_edges[c]
        posl = inv[d_c]
        order = np.argsort(posl, kind="stable")
        posl_s = posl[order]
        srcrow = pos_of[s_c[order]]
        first = np.concatenate([[True], posl_s[1:] != posl_s[:-1]])
        grp_start = np.flatnonzero(first)
        grp_len = np.diff(np.concatenate([grp_start, [len(posl_s)]]))
        j = np.arange(len(posl_s)) - np.repeat(grp_start, grp_len)
        cols = col_of_tile[posl_s // P] + j
        ell[c, posl_s % P, cols] = srcrow.astype(np.int32)
    return dict(S=S, TPC=TPC, SPAD=SPAD, K=K, L=L, ell=ell, perms=perms,
                col_of_tile=col_of_tile)


def _gather_reduce(nc, pools, meta, tabT, ell_sb, W, agg_sink):
    K, TPC, cot = meta["K"], meta["TPC"], meta["col_of_tile"]
    ellp, aggp = pools
    for t in range(TPC):
        Kt = int(K[t])
        a = aggp.tile([P, W], F32, tag=f"agg{W}")
        if Kt == 0:
            nc.vector.memset(a[:], 0.0)
            agg_sink(t, a)
            continue
        eb = ellp.tile([P, Kt * W], F32, tag=f"ell{W}")
        c0 = int(cot[t])
        for j in range(Kt):
            nc.gpsimd.indirect_dma_start(
                out=eb[:, j * W:(j + 1) * W],
                out_offset=None,
                in_=tabT[:, :],
                in_offset=bass.IndirectOffsetOnAxis(
                    ap=ell_sb[:, c0 + j:c0 + j + 1], axis=0),
            )
        nc.vector.tensor_reduce(
            out=a[:],
            in_=eb[:].rearrange("p (k w) -> p w k", k=Kt),
            op=mybir.AluOpType.add,
            axis=mybir.AxisListType.X,
        )
        agg_sink(t, a)


def build_layer1(meta):
    S, TPC, SPAD, L = meta["S"], meta["TPC"], meta["SPAD"], meta["L"]
    NTAB = N_CORES * SPAD
    CHUNK = 512
    NCH = (SPAD + CHUNK - 1) // CHUNK

    nc = bacc.Bacc()
    tab1 = nc.dram_tensor("tab1", [NTAB + 1, 64], F32, kind="ExternalInput")
    zT = nc.dram_tensor("zT", [64, SPAD], F32, kind="ExternalInput")
    ellix = nc.dram_tensor("ellix", [P, L], I32, kind="ExternalInput")
    Wr1 = nc.dram_tensor("Wr1", [64, 32], F32, kind="ExternalInput")
    Wo1 = nc.dram_tensor("Wo1", [64, 32], F32, kind="ExternalInput")
    b1 = nc.dram_tensor("b1", [32, 1], F32, kind="ExternalInput")
    Wr2 = nc.dram_tensor("Wr2", [32, 16], F32, kind="ExternalInput")
    h1T_o = nc.dram_tensor("h1T", [32, SPAD], F32, kind="ExternalOutput")
    y2_o = nc.dram_tensor("y2", [P, TPC * 16], F32, kind="ExternalOutput")

    with tile.TileContext(nc) as tc:
        with (
            tc.tile_pool(name="big", bufs=1) as big,
            tc.tile_pool(name="ell", bufs=3) as ellp,
            tc.tile_pool(name="agg", bufs=3) as aggp,
            tc.tile_pool(name="ps", bufs=2, space="PSUM") as psp,
            tc.tile_pool(name="pst", bufs=3, space="PSUM") as pstp,
            tc.tile_pool(name="sm", bufs=1) as sm,
        ):
            zT_sb = big.tile([64, SPAD], F32, tag="zT")
            ell_sb = big.tile([P, L], I32, tag="ellix")
            agg1T = big.tile([64, SPAD], F32, tag="agg1T")
            h1T = big.tile([32, SPAD], F32, tag="h1T")
            y2s = big.tile([P, TPC * 16], F32, tag="y2s")
            nc.sync.dma_start(out=zT_sb[:], in_=zT[:])
            nc.sync.dma_start(out=ell_sb[:], in_=ellix[:])
            w_rel1 = sm.tile([64, 32], F32, tag="wr1")
            w_root1 = sm.tile([64, 32], F32, tag="wo1")
            b1_sb = sm.tile([32, 1], F32, tag="b1")
            w_rel2 = sm.tile([32, 16], F32, tag="wr2")
            ident = sm.tile([P, P], F32, tag="id")
            nc.sync.dma_start(out=w_rel1[:], in_=Wr1[:])
            nc.sync.dma_start(out=w_root1[:], in_=Wo1[:])
            nc.sync.dma_start(out=b1_sb[:], in_=b1[:])
            nc.sync.dma_start(out=w_rel2[:], in_=Wr2[:])
            make_identity(nc, ident[:])

            def sink(t, a):
                pst = pstp.tile([64, P], F32, tag="pst")
                nc.tensor.transpose(out=pst[:], in_=a[:], identity=ident[:])
                nc.scalar.copy(out=agg1T[:, t * P:(t + 1) * P], in_=pst[:])

            _gather_reduce(nc, (ellp, aggp), meta, tab1, ell_sb, 64, sink)

            for ch in range(NCH):
                c0, c1 = ch * CHUNK, min((ch + 1) * CHUNK, SPAD)
                ph = psp.tile([32, CHUNK], F32, tag="ph1")
                nc.tensor.matmul(out=ph[:, :c1 - c0], lhsT=w_rel1[:], rhs=agg1T[:, c0:c1], start=True, stop=False)
                nc.tensor.matmul(out=ph[:, :c1 - c0], lhsT=w_root1[:], rhs=zT_sb[:, c0:c1], start=False, stop=True)
                nc.scalar.activation(out=h1T[:, c0:c1], in_=ph[:, :c1 - c0],
                                     func=mybir.ActivationFunctionType.Relu, bias=b1_sb[:])
            for t in range(TPC):
                py = psp.tile([P, 16], F32, tag="py2")
                nc.tensor.matmul(out=py[:], lhsT=h1T[:, t * P:(t + 1) * P], rhs=w_rel2[:], start=True, stop=True)
                nc.scalar.copy(out=y2s[:, t * 16:(t + 1) * 16], in_=py[:])
            nc.sync.dma_start(out=h1T_o[:, :], in_=h1T[:])
            nc.sync.dma_start(out=y2_o[:, :], in_=y2s[:])
    nc.compile()
    return nc


def build_layer2(meta):
    S, TPC, SPAD, L = meta["S"], meta["TPC"], meta["SPAD"], meta["L"]
    NTAB = N_CORES * SPAD
    CHUNK = 512
    NCH = (SPAD + CHUNK - 1) // CHUNK

    nc = bacc.Bacc()
    tab2 = nc.dram_tensor("tab2", [NTAB + 1, 16], F32, kind="ExternalInput")
    h1T_i = nc.dram_tensor("h1Ti", [32, SPAD], F32, kind="ExternalInput")
    ellix = nc.dram_tensor("ellix", [P, L], I32, kind="ExternalInput")
    Wo2 = nc.dram_tensor("Wo2", [32, 16], F32, kind="ExternalInput")
    b2 = nc.dram_tensor("b2", [16, 1], F32, kind="ExternalInput")
    Wr3 = nc.dram_tensor("Wr3", [16, 2], F32, kind="ExternalInput")
    h2T_o = nc.dram_tensor("h2T", [16, SPAD], F32, kind="ExternalOutput")
    y3_o = nc.dram_tensor("y3", [P, TPC * 2], F32, kind="ExternalOutput")

    with tile.TileContext(nc) as tc:
        with (
            tc.tile_pool(name="big", bufs=1) as big,
            tc.tile_pool(name="ell", bufs=3) as ellp,
            tc.tile_pool(name="agg", bufs=3) as aggp,
            tc.tile_pool(name="ps", bufs=2, space="PSUM") as psp,
            tc.tile_pool(name="pst", bufs=3, space="PSUM") as pstp,
            tc.tile_pool(name="sm", bufs=1) as sm,
        ):
            h1T = big.tile([32, SPAD], F32, tag="h1T")
            ell_sb = big.tile([P, L], I32, tag="ellix")
            agg2T = big.tile([16, SPAD], F32, tag="agg2T")
            h2T = big.tile([16, SPAD], F32, tag="h2T")
            y3s = big.tile([P, TPC * 2], F32, tag="y3s")
            nc.sync.dma_start(out=h1T[:], in_=h1T_i[:])
            nc.sync.dma_start(out=ell_sb[:], in_=ellix[:])
            w_root2 = sm.tile([32, 16], F32, tag="wo2")
            b2_sb = sm.tile([16, 1], F32, tag="b2")
            w_rel3 = sm.tile([16, 2], F32, tag="wr3")
            ident = sm.tile([P, P], F32, tag="id")
            nc.sync.dma_start(out=w_root2[:], in_=Wo2[:])
            nc.sync.dma_start(out=b2_sb[:], in_=b2[:])
            nc.sync.dma_start(out=w_rel3[:], in_=Wr3[:])
            make_identity(nc, ident[:])

            def sink(t, a):
                pst = pstp.tile([16, P], F32, tag="pst")
                nc.tensor.transpose(out=pst[:], in_=a[:], identity=ident[:])
                nc.scalar.copy(out=agg2T[:, t * P:(t + 1) * P], in_=pst[:])

            _gather_reduce(nc, (ellp, aggp), meta, tab2, ell_sb, 16, sink)

            for ch in range(NCH):
                c0, c1 = ch * CHUNK, min((ch + 1) * CHUNK, SPAD)
                ph = psp.tile([16, CHUNK], F32, tag="ph2")
                nc.tensor.matmul(out=ph[:, :c1 - c0], lhsT=w_root2[:], rhs=h1T[:, c0:c1], start=True, stop=True)
                nc.vector.tensor_add(out=agg2T[:, c0:c1], in0=agg2T[:, c0:c1], in1=ph[:, :c1 - c0])
                nc.scalar.activation(out=h2T[:, c0:c1], in_=agg2T[:, c0:c1],
                                     func=mybir.ActivationFunctionType.Relu, bias=b2_sb[:])
            for t in range(TPC):
                py = psp.tile([P, 2], F32, tag="py3")
                nc.tensor.matmul(out=py[:], lhsT=h2T[:, t * P:(t + 1) * P], rhs=w_rel3[:], start=True, stop=True)
                nc.scalar.copy(out=y3s[:, t * 2:(t + 1) * 2], in_=py[:])
            nc.sync.dma_start(out=h2T_o[:, :], in_=h2T[:])
            nc.sync.dma_start(out=y3_o[:, :], in_=y3s[:])
    nc.compile()
    return nc


def build_layer3(meta):
    S, TPC, SPAD, L = meta["S"], meta["TPC"], meta["SPAD"], meta["L"]
    NTAB = N_CORES * SPAD

    nc = bacc.Bacc()
    tab3 = nc.dram_tensor("tab3", [NTAB + 1, 2], F32, kind="ExternalInput")
    h2T_i = nc.dram_tensor("h2Ti", [16, SPAD], F32, kind="ExternalInput")
    ellix = nc.dram_tensor("ellix", [P, L], I32, kind="ExternalInput")
    Wo3 = nc.dram_tensor("Wo3", [16, 2], F32, kind="ExternalInput")
    b3bc = nc.dram_tensor("b3bc", [P, 2], F32, kind="ExternalInput")
    out_probs = nc.dram_tensor("out_probs", [P, TPC * 2], F32, kind="ExternalOutput")

    with tile.TileContext(nc) as tc:
        with (
            tc.tile_pool(name="big", bufs=1) as big,
            tc.tile_pool(name="ell", bufs=3) as ellp,
            tc.tile_pool(name="agg", bufs=3) as aggp,
            tc.tile_pool(name="ps", bufs=2, space="PSUM") as psp,
            tc.tile_pool(name="sm", bufs=1) as sm,
        ):
            h2T = big.tile([16, SPAD], F32, tag="h2T")
            ell_sb = big.tile([P, L], I32, tag="ellix")
            out3 = big.tile([P, TPC * 2], F32, tag="out3")
            ex = big.tile([P, TPC * 2], F32, tag="ex")
            nc.sync.dma_start(out=h2T[:], in_=h2T_i[:])
            nc.sync.dma_start(out=ell_sb[:], in_=ellix[:])
            w_root3 = sm.tile([16, 2], F32, tag="wo3")
            b3_sb = sm.tile([P, 2], F32, tag="b3")
            mx = sm.tile([P, TPC], F32, tag="mx")
            sme = sm.tile([P, TPC], F32, tag="sme")
            nc.sync.dma_start(out=w_root3[:], in_=Wo3[:])
            nc.sync.dma_start(out=b3_sb[:], in_=b3bc[:])

            def sink(t, a):
                po = psp.tile([P, 2], F32, tag="po3")
                nc.tensor.matmul(out=po[:], lhsT=h2T[:, t * P:(t + 1) * P], rhs=w_root3[:], start=True, stop=True)
                nc.vector.tensor_add(out=out3[:, t * 2:(t + 1) * 2], in0=a[:], in1=po[:])

            _gather_reduce(nc, (ellp, aggp), meta, tab3, ell_sb, 2, sink)

            nc.vector.tensor_tensor(
                out=out3[:], in0=out3[:],
                in1=b3_sb[:, None, :].to_broadcast([P, TPC, 2]),
                op=mybir.AluOpType.add)
            nc.vector.tensor_reduce(out=mx[:], in_=out3[:].rearrange("p (t w) -> p t w", w=2),
                                    op=mybir.AluOpType.max, axis=mybir.AxisListType.X)
            nc.vector.tensor_tensor(
                out=out3[:], in0=out3[:],
                in1=mx[:, :, None].to_broadcast([P, TPC, 2]),
                op=mybir.AluOpType.subtract)
            nc.scalar.activation(out=ex[:], in_=out3[:], func=mybir.ActivationFunctionType.Exp)
            nc.vector.tensor_reduce(out=sme[:], in_=ex[:].rearrange("p (t w) -> p t w", w=2),
                                    op=mybir.AluOpType.add, axis=mybir.AxisListType.X)
            nc.vector.reciprocal(out=sme[:], in_=sme[:])
            nc.vector.tensor_tensor(
                out=ex[:], in0=ex[:],
                in1=sme[:, :, None].to_broadcast([P, TPC, 2]),
                op=mybir.AluOpType.mult)
            nc.sync.dma_start(out=out_probs[:, :], in_=ex[:])
    nc.compile()
    return nc


def _stage_to_rows(stage, TPC, W):
    return stage.reshape(P, TPC, W).transpose(1, 0, 2).reshape(-1, W)


def kernel(z, edge_index, W_rel1, W_root1, b1, W_rel2, W_root2, b2,
           W_rel3, W_root3, b3):
    global LAST_EXEC_TIME_NS
    trace = _install_ntff_hook()

    n_nodes = z.shape[0]
    meta = preprocess(edge_index, n_nodes)
    S, TPC, SPAD, L = meta["S"], meta["TPC"], meta["SPAD"], meta["L"]
    NTAB = N_CORES * SPAD
    z = np.asarray(z, dtype=np.float32)

    nc1 = build_layer1(meta)
    nc2 = build_layer2(meta)
    nc3 = build_layer3(meta)

    tab1 = np.zeros((NTAB + 1, 64), np.float32)
    for c in range(N_CORES):
        tab1[c * SPAD:c * SPAD + S] = z[c * S:(c + 1) * S][meta["perms"][c]]

    wk = {
        "W_rel1": np.asarray(W_rel1, np.float32), "W_root1": np.asarray(W_root1, np.float32),
        "b1": np.asarray(b1, np.float32), "W_rel2": np.asarray(W_rel2, np.float32),
        "W_root2": np.asarray(W_root2, np.float32), "b2": np.asarray(b2, np.float32),
        "W_rel3": np.asarray(W_rel3, np.float32), "W_root3": np.asarray(W_root3, np.float32),
        "b3": np.asarray(b3, np.float32),
    }
    total_ns = 0

    maps1 = []
    for c in range(N_CORES):
        zTc = np.zeros((64, SPAD), np.float32)
        zTc[:, :S] = tab1[c * SPAD:c * SPAD + S].T
        maps1.append({"tab1": tab1, "zT": zTc, "ellix": meta["ell"][c],
                      "Wr1": wk["W_rel1"], "Wo1": wk["W_root1"],
                      "b1": wk["b1"].reshape(32, 1), "Wr2": wk["W_rel2"]})
    r1 = run_bass_kernel_spmd(nc1, maps1, core_ids=list(range(N_CORES)), trace=trace)
    if r1.exec_time_ns:
        total_ns += r1.exec_time_ns

    tab2 = np.zeros((NTAB + 1, 16), np.float32)
    for c in range(N_CORES):
        rows = _stage_to_rows(np.asarray(r1.results[c]["y2"], np.float32), TPC, 16)
        rows[S:] = 0.0
        tab2[c * SPAD:(c + 1) * SPAD] = rows
    maps2 = [{"tab2": tab2, "h1Ti": np.asarray(r1.results[c]["h1T"], np.float32),
              "ellix": meta["ell"][c], "Wo2": wk["W_root2"],
              "b2": wk["b2"].reshape(16, 1), "Wr3": wk["W_rel3"]}
             for c in range(N_CORES)]
    r2 = run_bass_kernel_spmd(nc2, maps2, core_ids=list(range(N_CORES)), trace=trace)
    if r2.exec_time_ns:
        total_ns += r2.exec_time_ns

    tab3 = np.zeros((NTAB + 1, 2), np.float32)
    for c in range(N_CORES):
        rows = _stage_to_rows(np.asarray(r2.results[c]["y3"], np.float32), TPC, 2)
        rows[S:] = 0.0
        tab3[c * SPAD:(c + 1) * SPAD] = rows
    maps3 = [{"tab3": tab3, "h2Ti": np.asarray(r2.results[c]["h2T"], np.float32),
              "ellix": meta["ell"][c], "Wo3": wk["W_root3"],
              "b3bc": np.tile(wk["b3"].reshape(1, 2), (P, 1)).astype(np.float32)}
             for c in range(N_CORES)]
    r3 = run_bass_kernel_spmd(nc3, maps3, core_ids=list(range(N_CORES)), trace=trace)
    if r3.exec_time_ns:
        total_ns += r3.exec_time_ns

    out = np.empty((n_nodes, 2), np.float32)
    for c in range(N_CORES):
        probs = _stage_to_rows(np.asarray(r3.results[c]["out_probs"], np.float32), TPC, 2)
        shard = np.empty((S, 2), np.float32)
        shard[meta["perms"][c]] = probs[:S]
        out[c * S:(c + 1) * S] = shard

    LAST_EXEC_TIME_NS = total_ns if total_ns > 0 else None
    return out

